# revision 11
# baseline (speedup 1.0000x reference)
"""Trainium2 Bass kernel for nn_CalcRayColor (NeRF-style volume rendering).

Math (per ray, N_p=128 samples):
    a_i     = density_i * dists_i
    x_i     = exp(-a_i)                      # == 1 - alpha_i  (the +1e-10 in the
                                             #  reference is ~3e-10 relative, < f32 eps)
    P_i     = prod_{j<=i} x_j                # inclusive cumprod (transmittance past i)
    weight_i = alpha_i * trans_i = P_{i-1} - P_i
    rgb_res  = sum_i weight_i * rgb_i        (3 channels)
    depth    = sum_i weight_i * z_i
    acc      = sum_i weight_i = 1 - P_127    (telescoping)
    bg_alpha = 1 - acc = P_127

Sharding: pure data-parallel over rays; 65536 rays / 8 cores = 8192 per core.

On-core layout: rays on partitions. Each supertile covers RT = 128*G rays;
partition p holds rays t*RT + p*G + g (g in [0,G)), so every DMA moves
G*128 contiguous f32 (=4KB v G=8) per partition line.

Pipeline per supertile (all stock ops):
    DVE : a = den*dis ; per-group cumprod via tensor_tensor_scan ;
          w = P[:-1] - P[1:] ; fused mul+reduce dots (tensor_tensor_reduce)
    ACT : x = exp(-a) ; bg extraction
    DMA : 4 loads, 1 weight store (+1 stats store at the end)
"""

import numpy as np

import concourse.bass as bass
import concourse.mybir as mybir
from concourse.tile import TileContext
from concourse.bass_utils import run_bass_kernel_spmd

F32 = mybir.dt.float32
NP_ = 128          # samples per ray
N_CORES = 8
NR_FULL = 65536
NRC = NR_FULL // N_CORES   # rays per core


def build_nc(nrc=NRC, g=8):
    """Build the single-core Bass program (SPMD across cores via input maps)."""
    pt = 128                # partitions
    rt = pt * g             # rays per supertile
    t_count = nrc // rt
    assert nrc % rt == 0

    nc = bass.Bass(trn_type="TRN2")

    den = nc.declare_dram_parameter("den", [nrc, NP_], F32, isOutput=False)
    dis = nc.declare_dram_parameter("dis", [nrc, NP_], F32, isOutput=False)
    zv = nc.declare_dram_parameter("zv", [nrc, NP_], F32, isOutput=False)
    rgb = nc.declare_dram_parameter("rgb", [3, nrc, NP_], F32, isOutput=False)
    wout = nc.declare_dram_parameter("wout", [nrc, NP_], F32, isOutput=True)
    # stats[p, ch, t*g + gi] = stat for ray t*rt + p*g + gi
    # ch: 0..2 = rgb dot, 3 = depth dot, 4 = bg_alpha (= P_last)
    stats = nc.declare_dram_parameter("stats", [pt, 5, t_count * g], F32, isOutput=True)

    den_r = den[:].rearrange("(t p g) s -> t p (g s)", t=t_count, p=pt, g=g)
    dis_r = dis[:].rearrange("(t p g) s -> t p (g s)", t=t_count, p=pt, g=g)
    zv_r = zv[:].rearrange("(t p g) s -> t p (g s)", t=t_count, p=pt, g=g)
    rgb_r = rgb[:].rearrange("c (t p g) s -> t p c (g s)", t=t_count, p=pt, g=g)
    wout_r = wout[:].rearrange("(t p g) s -> t p (g s)", t=t_count, p=pt, g=g)

    mult = mybir.AluOpType.mult
    add = mybir.AluOpType.add
    bypass = mybir.AluOpType.bypass
    exp_fn = mybir.ActivationFunctionType.Exp

    with TileContext(nc) as tc:
        with (
            tc.tile_pool(name="io", bufs=3) as pool,
            tc.tile_pool(name="stat", bufs=1) as spool,
        ):
            stats_t = spool.tile([pt, 5, t_count * g], F32)
            for t in range(t_count):
                den_t = pool.tile([pt, g * NP_], F32)
                nc.sync.dma_start(out=den_t[:], in_=den_r[t])
                dis_t = pool.tile([pt, g * NP_], F32)
                nc.sync.dma_start(out=dis_t[:], in_=dis_r[t])
                zv_t = pool.tile([pt, g * NP_], F32)
                nc.sync.dma_start(out=zv_t[:], in_=zv_r[t])
                rgb_t = pool.tile([pt, 3, g * NP_], F32)
                nc.sync.dma_start(out=rgb_t[:], in_=rgb_r[t])

                a_t = pool.tile([pt, g * NP_], F32)
                nc.vector.tensor_mul(a_t[:], den_t[:], dis_t[:])
                x_t = pool.tile([pt, g * NP_], F32)
                nc.scalar.activation(x_t[:], a_t[:], exp_fn, scale=-1.0)

                # pf[p, gi, 0] = 1; pf[p, gi, 1:] = inclusive cumprod of x
                pf = pool.tile([pt, g, NP_ + 1], F32)
                nc.vector.memset(pf[:, :, 0:1], 1.0)
                for gi in range(g):
                    xg = x_t[:, gi * NP_ : (gi + 1) * NP_]
                    nc.vector.tensor_tensor_scan(
                        pf[:, gi, 1 : NP_ + 1], xg, xg, 1.0, op0=mult, op1=bypass
                    )

                w_t = pool.tile([pt, g, NP_], F32)
                nc.vector.tensor_sub(w_t[:], pf[:, :, 0:NP_], pf[:, :, 1 : NP_ + 1])
                nc.sync.dma_start(
                    out=wout_r[t], in_=w_t[:].rearrange("p g s -> p (g s)")
                )

                # Dot products: product pass + segmented reduce per channel.
                for ci in range(4):
                    if ci < 3:
                        src = rgb_t[:, ci, :]
                    else:
                        src = zv_t[:]
                    prod = pool.tile([pt, g * NP_], F32)
                    nc.vector.tensor_mul(
                        prod[:], w_t[:].rearrange("p g s -> p (g s)"), src
                    )
                    nc.vector.tensor_reduce(
                        stats_t[:, ci, t * g : (t + 1) * g],
                        prod[:].rearrange("p (g s) -> p g s", g=g),
                        axis=mybir.AxisListType.X,
                        op=add,
                    )

                # bg_alpha = P_last
                nc.scalar.copy(
                    stats_t[:, 4, t * g : (t + 1) * g],
                    pf[:, :, NP_ : NP_ + 1].rearrange("p g one -> p (g one)"),
                )

            nc.sync.dma_start(out=stats[:], in_=stats_t[:])
    return nc


def legalize_waits(nc, limit=1):
    """Split sync waits exceeding `limit` per instruction onto same-engine
    wait-carrier nops inserted immediately before the instruction.

    The walrus codegen in this container rejects instructions with more
    than one attached semaphore wait ("Too many sync wait commands") —
    the library pass that normally elides the extra waits (optimize_sems)
    is disabled (inc-6505). A nop carrying the excess waits right before
    the instruction on the same engine is semantically identical.
    """
    eng_builder = {
        mybir.EngineType.DVE: nc.vector,
        mybir.EngineType.Activation: nc.scalar,
        mybir.EngineType.PE: nc.tensor,
        mybir.EngineType.Pool: nc.gpsimd,
        mybir.EngineType.SP: nc.sync,
    }
    n_split = 0
    n_nops = 0
    _dummy_sem_cm = nc.semaphore("wait_legalize_dummy")
    _dummy_sem = _dummy_sem_cm.__enter__()
    for f in nc.m.functions:
        blocks = list(f.blocks)
        # collect target instructions per block first
        plans = []  # (block, list of (index, inst))
        for blk in blocks:
            insts = list(blk.instructions)
            targets = []
            for idx, inst in enumerate(insts):
                si = inst.sync_info
                if si is not None and len(si.on_wait) > limit:
                    targets.append(idx)
            if targets:
                plans.append((blk, insts, targets))
        for blk, insts, targets in plans:
            new_list = []
            nop_carriers = {}  # idx -> list of raw nop instructions
            for idx in targets:
                inst = insts[idx]
                si = inst.sync_info
                waits = list(si.on_wait)
                keep = waits[:limit]
                excess = waits[limit:]
                carriers = []
                import bass_rust

                for w in excess:
                    carrier = eng_builder[inst.engine].wait_ge(_dummy_sem, 0)
                    raw = carrier.ins
                    raw.sync_info = bass_rust.SyncInfo(
                        on_wait=[w], on_update=[]
                    )
                    try:
                        raw.bass_nofuse = True
                    except Exception:
                        pass
                    carriers.append(raw)
                    n_nops += 1
                si.on_wait = keep
                inst.sync_info = si
                nop_carriers[idx] = carriers
                n_split += 1
            # nops were appended to some current block; remove them there
            carrier_names = {
                r.name for lst in nop_carriers.values() for r in lst
            }
            for b2 in f.blocks:
                li = list(b2.instructions)
                if any(i.name in carrier_names for i in li):
                    b2.instructions = [
                        i for i in li if i.name not in carrier_names
                    ]
            for idx, inst in enumerate(insts):
                if idx in nop_carriers:
                    new_list.extend(nop_carriers[idx])
                new_list.append(inst)
            blk.instructions = new_list
    _dummy_sem_cm.__exit__(None, None, None)
    return {"split": n_split, "nops": n_nops}


_NC_CACHE = {}


def _get_nc():
    key = (NRC, 8)
    if key not in _NC_CACHE:
        nc = build_nc(NRC, 8)
        legalize_waits(nc)
        _NC_CACHE[key] = nc
    return _NC_CACHE[key]


def _decode_stats(arr, nrc=NRC, g=8):
    """[128, 5, T*g] -> [5, nrc] with ray r = t*128*g + p*g + gi."""
    pt = 128
    t_count = nrc // (pt * g)
    a = arr.reshape(pt, 5, t_count, g)
    return np.ascontiguousarray(a.transpose(1, 2, 0, 3)).reshape(5, nrc)


def _make_in_maps(batch_rgb, batch_density, batch_dists, batch_z_vals):
    den = np.ascontiguousarray(np.asarray(batch_density, np.float32)[0, 0])
    dis = np.ascontiguousarray(np.asarray(batch_dists, np.float32)[0, 0])
    zv = np.ascontiguousarray(np.asarray(batch_z_vals, np.float32)[0])
    rgb = np.ascontiguousarray(np.asarray(batch_rgb, np.float32)[0])  # [3, NR, NP]

    in_maps = []
    for c in range(N_CORES):
        sl = slice(c * NRC, (c + 1) * NRC)
        in_maps.append(
            {
                "den": np.ascontiguousarray(den[sl]),
                "dis": np.ascontiguousarray(dis[sl]),
                "zv": np.ascontiguousarray(zv[sl]),
                "rgb": np.ascontiguousarray(rgb[:, sl]),
            }
        )
    return in_maps


def _assemble(results):
    weight = np.concatenate([r["wout"] for r in results], axis=0)
    weight = weight.reshape(1, 1, NR_FULL, NP_).astype(np.float32)
    sv = np.concatenate(
        [_decode_stats(np.asarray(r["stats"])) for r in results], axis=1
    )  # [5, NR]
    rgb_res = sv[0:3][None].astype(np.float32)          # [1, 3, NR]
    depth_res = sv[3][None, None].astype(np.float32)    # [1, 1, NR]
    bg_alpha = sv[4][None, None].astype(np.float32)     # [1, 1, NR]
    return (rgb_res, bg_alpha, depth_res, weight)


def kernel(fg_vps, batch_rgb, batch_density, batch_dists, batch_z_vals):
    del fg_vps  # unused by the reference computation
    in_maps = _make_in_maps(batch_rgb, batch_density, batch_dists, batch_z_vals)
    nc = _get_nc()
    res = run_bass_kernel_spmd(nc, in_maps, list(range(N_CORES)))
    return _assemble(res.results)
